# revision 5
# baseline (speedup 1.0000x reference)
"""CRTN middle_l query construction as a pure-DMA Bass kernel on 8 TRN2 cores.

Hybrid-layout mega-DMA design — 4 DMAs per core.

Math (from the reference):
    query_base = concat([neighbor_mem[-1], wise_inputs], axis=0)   # (256, B, H)
    query[i, j] = query_base[i + j + 1]                            # (S, S, B, H)

For fixed i, query[i] = query_base[i+1 : i+129] is one contiguous 8 MB slab —
the whole problem is memory-bound replication: 16 MB of source fanned out to
1 GiB of output. Sharding: data-parallel over i (16 output rows per core;
core k stages query_base rows [16k+1, 16k+144)).

SBUF layout: rows 0..127 stored WHOLE (64 KB per partition, partition = local
row index); tail rows 128..142 chunked into 120 x 8 KB at byte offset 64 KB
(partition = 8*(row-128) + chunk).

Output rows padded 128 -> 143 slots of 64 KB (flat stride RSTRIDE = 143*16384
elems; the m-dim stride is MSTRIDE = RSTRIDE - 16384 because window m starts
one row later per m). Two mega-DMAs write everything:

  MEGA-1: (p:128, m:16, o:16384)  flat = m*MSTRIDE + p*16384  (128 MB,
          64 KB descriptors, uniform across all engines). Positions with
          p < m land in the previous row's pad -> discarded by host.
  MEGA-2: (p:120, m:15, o:2048)   flat = m*MSTRIDE + 128*16384 + p*2048
          (tail rows; chunk flat offset 2048*p is linear in p because
          (p//8)*16384 + (p%8)*2048 = 2048*p). Overruns land in pad.

Every in-row position is written exactly once => no WAW hazards.
Total per core: S1 (8 MB) + S2 (0.94 MB) staging + 2 megas = 4 DMAs."""

import numpy as np

import concourse.bacc as bacc
import concourse.bass as bass
import concourse.mybir as mybir
import concourse.tile as tile
from concourse.bass_utils import run_bass_kernel_spmd

NEI_LEN = 128
S = 128
B = 16
H = 1024
N_CORES = 8
ROWS_PER_CORE = S // N_CORES          # 16
IN_ROWS = ROWS_PER_CORE + S - 1       # 143
ROW_ELEMS = B * H                     # 16384 f32 = 64 KB per source row
CH = 2048                             # tail chunk: 8 KB
ROWP = 143                            # padded row length in 64 KB slots
RSTRIDE = ROWP * ROW_ELEMS            # 2342912 elems per padded output row
MSTRIDE = RSTRIDE - ROW_ELEMS         # 2326528
OUT_ELEMS = MSTRIDE + 128 * ROW_ELEMS + 15 * MSTRIDE  # 39321600

LAST_EXEC_NS = None
_nc_cache = None


def _build_nc(repeats: int = 1) -> bass.Bass:
    nc = bacc.Bacc("TRN2", target_bir_lowering=False, debug=False)
    qb = nc.dram_tensor(
        "qb", [IN_ROWS, ROW_ELEMS], mybir.dt.float32, kind="ExternalInput"
    )
    out = nc.dram_tensor("out", [OUT_ELEMS], mybir.dt.float32, kind="ExternalOutput")
    out_flat = out.ap()
    qb_flat = qb.ap().rearrange("r o -> (r o)")
    with tile.TileContext(nc) as tc:
        with tc.tile_pool(name="stage", bufs=min(repeats, 2)) as pool:
            for _ in range(repeats):
                buf = pool.tile([128, ROW_ELEMS + CH], mybir.dt.float32)
                # S1: rows 0..127, one row per partition
                nc.sync.dma_start(out=buf[:, 0:ROW_ELEMS], in_=qb.ap()[0:128])
                # S2: tail rows 128..142 as 120 chunks of 8 KB
                nc.scalar.dma_start(
                    out=buf[0:120, ROW_ELEMS : ROW_ELEMS + CH],
                    in_=qb_flat[128 * ROW_ELEMS : 143 * ROW_ELEMS].rearrange(
                        "(p o) -> p o", p=120
                    ),
                )
                # MEGA-1: body rows for all 16 windows
                mega1 = (
                    out_flat[0 : 16 * MSTRIDE]
                    .rearrange("(m x) -> m x", m=16)[:, 0 : 128 * ROW_ELEMS]
                    .rearrange("m (p o) -> p m o", p=128)
                )
                nc.sync.dma_start(
                    out=mega1,
                    in_=buf[:, 0:ROW_ELEMS].unsqueeze(1).broadcast_to(
                        [128, 16, ROW_ELEMS]
                    ),
                )
                # MEGA-2: tail chunks for windows m in [1,16)
                base2 = MSTRIDE + 128 * ROW_ELEMS
                mega2 = (
                    out_flat[base2 : base2 + 15 * MSTRIDE]
                    .rearrange("(m x) -> m x", m=15)[:, 0 : 120 * CH]
                    .rearrange("m (p o) -> p m o", p=120)
                )
                nc.scalar.dma_start(
                    out=mega2,
                    in_=buf[0:120, ROW_ELEMS : ROW_ELEMS + CH]
                    .unsqueeze(1)
                    .broadcast_to([120, 15, CH]),
                )
    nc.compile()
    return nc


def kernel(neighbor_mem: np.ndarray, wise_inputs: np.ndarray) -> np.ndarray:
    global _nc_cache, LAST_EXEC_NS
    assert neighbor_mem.shape == (13, NEI_LEN, B, H), neighbor_mem.shape
    assert wise_inputs.shape == (S, B, H), wise_inputs.shape

    qb_full = np.concatenate(
        [
            np.asarray(neighbor_mem[-1], dtype=np.float32).reshape(NEI_LEN, ROW_ELEMS),
            np.asarray(wise_inputs, dtype=np.float32).reshape(S, ROW_ELEMS),
        ],
        axis=0,
    )  # (256, 16384)

    in_maps = [
        {"qb": qb_full[ROWS_PER_CORE * k + 1 : ROWS_PER_CORE * k + 1 + IN_ROWS]}
        for k in range(N_CORES)
    ]

    if _nc_cache is None:
        _nc_cache = _build_nc()

    res = run_bass_kernel_spmd(_nc_cache, in_maps, core_ids=list(range(N_CORES)))
    LAST_EXEC_NS = res.exec_time_ns

    parts = []
    for r in res.results:
        o = r["out"][: 16 * RSTRIDE].reshape(ROWS_PER_CORE, ROWP, ROW_ELEMS)
        parts.append(o[:, :S, :].reshape(ROWS_PER_CORE, S, B, H))
    return np.concatenate(parts, axis=0)
